# revision 7
# baseline (speedup 1.0000x reference)
"""Trainium2 Bass kernel for nn_MultiHeadAttention_30734785970264.

Linear ("elu+1") attention, faithful to the reference:
  q/k/v projections, kf = (elu(k)+1)*km, vm = v*km,
  C[b,h]   = sum_s (S-s) * sum_d(kf*vm)          (global over s)
  Kc       = per-head cumsum of kf over feature dim
  Z[b,h,s] = 1/(sum_d qf*Kc + eps)
  out      = qf * C * Z ;  hidden = out @ Wo.T
Returns (hidden, k, v) like the reference.

Sharding: batch x sequence-half across 8 cores (all heads local); the only
cross-core dependency is the [16]-float C partial, AllReduce'd between the
two cores of each batch. Activations are fed feature-major (host transpose)
so every GEMM contracts on the partition dim; all GEMMs run as float32r.
"""

import sys

try:
    import concourse.bass as bass  # noqa: F401
except ImportError:
    sys.path.insert(0, "/opt/trn_rl_repo")

import numpy as np
from contextlib import ExitStack

import concourse.bass as bass
import concourse.mybir as mybir
import concourse.tile as tile
from concourse import bacc
from concourse.alu_op_type import AluOpType
from concourse.bass_utils import run_bass_kernel_spmd

F32 = mybir.dt.float32
F32R = mybir.dt.float32r

B, S, H, DH, D = 4, 4096, 16, 64, 1024
EPS = 1e-6
S_HALF = S // 2          # 2048 rows per core
CS = 512                 # s-chunk (f32 moving-dim max)
NCH = S_HALF // CS       # 4 chunks
NMT = D // 128           # 8 feature m-tiles
NKT = D // 128           # 8 contraction k-tiles
NST = CS // 128          # 4 s-subtiles per chunk

_CACHE = {}


def _build_nc():
    nc = bacc.Bacc()

    xq = nc.declare_dram_parameter("xq", [D, S_HALF], F32R, isOutput=False)
    xk = nc.declare_dram_parameter("xk", [D, S_HALF], F32R, isOutput=False)
    xv = nc.declare_dram_parameter("xv", [D, S_HALF], F32R, isOutput=False)
    wq = nc.declare_dram_parameter("wq", [D, D], F32R, isOutput=False)
    wk = nc.declare_dram_parameter("wk", [D, D], F32R, isOutput=False)
    wv = nc.declare_dram_parameter("wv", [D, D], F32R, isOutput=False)
    wo = nc.declare_dram_parameter("wo", [D, D], F32R, isOutput=False)
    km = nc.declare_dram_parameter("km", [1, S_HALF], F32, isOutput=False)
    wvec = nc.declare_dram_parameter("wvec", [1, S_HALF], F32, isOutput=False)
    lblock = nc.declare_dram_parameter("lblock", [128, 128], F32R, isOutput=False)
    hsum = nc.declare_dram_parameter("hsum", [128, 128], F32R, isOutput=False)
    eall = nc.declare_dram_parameter("eall", [16, D], F32R, isOutput=False)
    ident = nc.declare_dram_parameter("ident", [128, 128], F32R, isOutput=False)

    k_out = nc.declare_dram_parameter("k_out", [H, S_HALF, DH], F32, isOutput=True)
    v_out = nc.declare_dram_parameter("v_out", [H, S_HALF, DH], F32, isOutput=True)
    h_out = nc.declare_dram_parameter("h_out", [S_HALF, D], F32, isOutput=True)

    kc_s = nc.dram_tensor("kc_s", [D, S_HALF], F32)
    cc_in = nc.dram_tensor("cc_in", [16, 1], F32)
    cc_out = nc.dram_tensor("cc_out", [16, 1], F32)

    with tile.TileContext(nc) as tc, ExitStack() as ctx:
        wpool = ctx.enter_context(tc.tile_pool(name="wpool", bufs=2))
        xpool = ctx.enter_context(tc.tile_pool(name="xpool", bufs=3))
        mpool = ctx.enter_context(tc.tile_pool(name="mpool", bufs=2))
        spool = ctx.enter_context(tc.tile_pool(name="spool", bufs=2))
        cpool = ctx.enter_context(tc.tile_pool(name="cpool", bufs=1))
        pp_proj = ctx.enter_context(tc.tile_pool(name="pp_proj", bufs=3, space="PSUM"))
        pp_aux = ctx.enter_context(tc.tile_pool(name="pp_aux", bufs=2, space="PSUM"))
        pp_s16 = ctx.enter_context(tc.tile_pool(name="pp_s16", bufs=2, space="PSUM"))

        # ---- constants / broadcasts ----
        lb_sb = cpool.tile([128, 128], F32R, tag="lb")
        nc.sync.dma_start(out=lb_sb, in_=lblock[:, :])
        hs_sb = cpool.tile([128, 128], F32R, tag="hs")
        nc.sync.dma_start(out=hs_sb, in_=hsum[:, :])
        ea_sb = cpool.tile([16, D], F32R, tag="ea")
        nc.sync.dma_start(out=ea_sb, in_=eall[:, :])
        id_sb = cpool.tile([128, 128], F32R, tag="id")
        nc.sync.dma_start(out=id_sb, in_=ident[:, :])
        km_sb = cpool.tile([128, S_HALF], F32, tag="km")
        nc.sync.dma_start(
            out=km_sb, in_=bass.AP(tensor=km.tensor if hasattr(km, "tensor") else km,
                                   offset=0, ap=[[0, 128], [1, S_HALF]]))
        cacc = cpool.tile([16, 1], F32, tag="cacc")
        nc.vector.memset(cacc, 0.0)

        def load_w(param):
            t = wpool.tile([128, NKT, D], F32R, tag="w")
            nc.sync.dma_start(out=t, in_=param.rearrange("(kt p) m -> p kt m", p=128))
            return t

        wk_sb = load_w(wk)
        wv_sb = load_w(wv)

        # ================= phase 1: k/v proj, kf, t, C, Kc =================
        for c in range(NCH):
            s0 = c * CS
            xk_c = xpool.tile([128, NKT, CS], F32R, tag="x")
            nc.sync.dma_start(out=xk_c, in_=xk[:, s0:s0 + CS].rearrange("(kt p) s -> p kt s", p=128))
            xv_c = xpool.tile([128, NKT, CS], F32R, tag="x")
            nc.sync.dma_start(out=xv_c, in_=xv[:, s0:s0 + CS].rearrange("(kt p) s -> p kt s", p=128))

            t_ps = pp_s16.tile([16, CS], F32, tag="s16")

            for mt in range(NMT):
                msl = bass.ts(mt, 128)
                # k projection
                pk = pp_proj.tile([128, CS], F32, tag="proj")
                for kt in range(NKT):
                    nc.tensor.matmul(pk, wk_sb[:, kt, msl], xk_c[:, kt, :],
                                     start=(kt == 0), stop=(kt == NKT - 1))
                kT = mpool.tile([128, CS], F32R, tag="kT")
                nc.scalar.copy(kT, pk)
                # v projection
                pv = pp_proj.tile([128, CS], F32, tag="proj")
                for kt in range(NKT):
                    nc.tensor.matmul(pv, wv_sb[:, kt, msl], xv_c[:, kt, :],
                                     start=(kt == 0), stop=(kt == NKT - 1))
                vT = mpool.tile([128, CS], F32R, tag="vT")
                nc.scalar.copy(vT, pv)

                # kf = (elu(k)+1)*km ; vm = v*km
                tmp = mpool.tile([128, CS], F32, tag="tmp")
                nc.vector.tensor_scalar_min(tmp, kT, 0.0)
                ex = mpool.tile([128, CS], F32, tag="ex")
                nc.scalar.activation(ex, tmp, mybir.ActivationFunctionType.Exp)
                kf0 = mpool.tile([128, CS], F32, tag="tmp")
                nc.vector.scalar_tensor_tensor(kf0, kT, 0.0, ex,
                                               op0=AluOpType.max, op1=AluOpType.add)
                kf = mpool.tile([128, CS], F32R, tag="kf")
                nc.vector.tensor_tensor(out=kf, in0=kf0, in1=km_sb[:, s0:s0 + CS],
                                        op=AluOpType.mult)
                vm = mpool.tile([128, CS], F32R, tag="vm")
                nc.vector.tensor_tensor(out=vm, in0=vT, in1=km_sb[:, s0:s0 + CS],
                                        op=AluOpType.mult)
                P = mpool.tile([128, CS], F32R, tag="P")
                nc.vector.tensor_tensor(out=P, in0=kf, in1=vm, op=AluOpType.mult)
                # t[h,s] head-sum (accumulated across m-tiles)
                nc.tensor.matmul(t_ps, hs_sb[:, mt * 16:(mt + 1) * 16], P,
                                 start=(mt == 0), stop=(mt == NMT - 1))
                # Kc = per-head prefix sum of kf along features
                pkc = pp_aux.tile([128, CS], F32, tag="aux")
                nc.tensor.matmul(pkc, lb_sb, kf, start=True, stop=True)
                kc_sb = mpool.tile([128, CS], F32, tag="kc")
                nc.vector.tensor_copy(kc_sb, pkc)
                nc.sync.dma_start(out=kc_s[mt * 128:(mt + 1) * 128, s0:s0 + CS], in_=kc_sb)

                # s-major transposes of kT / vT -> k_out / v_out
                kstf = mpool.tile([128, NST, 128], F32, tag="kstf")
                vstf = mpool.tile([128, NST, 128], F32, tag="vstf")
                for sti in range(NST):
                    ssl = bass.ts(sti, 128)
                    ptr = pp_aux.tile([128, 128], F32R, tag="aux")
                    nc.tensor.transpose(ptr, kT[:, ssl], id_sb)
                    nc.vector.tensor_copy(kstf[:, sti, :], ptr)
                    ptr2 = pp_aux.tile([128, 128], F32R, tag="aux")
                    nc.tensor.transpose(ptr2, vT[:, ssl], id_sb)
                    nc.vector.tensor_copy(vstf[:, sti, :], ptr2)
                for hi in range(2):
                    h_idx = 2 * mt + hi
                    nc.sync.dma_start(
                        out=k_out[h_idx, s0:s0 + CS, :].rearrange("(a p) d -> p a d", p=128),
                        in_=kstf[:, :, hi * 64:(hi + 1) * 64])
                    nc.sync.dma_start(
                        out=v_out[h_idx, s0:s0 + CS, :].rearrange("(a p) d -> p a d", p=128),
                        in_=vstf[:, :, hi * 64:(hi + 1) * 64])

            # C partial accumulation: sum_s t[h,s] * wvec[s]
            wv16 = spool.tile([16, CS], F32, tag="wv16")
            nc.sync.dma_start(
                out=wv16, in_=bass.AP(tensor=wvec.tensor if hasattr(wvec, "tensor") else wvec,
                                      offset=s0, ap=[[0, 16], [1, CS]]))
            tmul = spool.tile([16, CS], F32, tag="tmul")
            nc.vector.tensor_tensor(out=tmul, in0=t_ps, in1=wv16,
                                    op=AluOpType.mult)
            cch = spool.tile([16, 1], F32, tag="cch")
            nc.vector.tensor_reduce(cch, tmul, axis=mybir.AxisListType.X, op=AluOpType.add)
            nc.vector.tensor_add(cacc, cacc, cch)

        # ---- AllReduce C across the two cores of this batch ----
        nc.sync.dma_start(out=cc_in[:, :], in_=cacc)
        nc.gpsimd.collective_compute(
            "AllReduce", AluOpType.add,
            replica_groups=[[0, 1], [2, 3], [4, 5], [6, 7]],
            ins=[cc_in.ap().opt()], outs=[cc_out.ap().opt()])
        c_sb = cpool.tile([16, 1], F32, tag="c_sb")
        nc.sync.dma_start(out=c_sb, in_=cc_out[:, :])

        wq_sb = load_w(wq)
        wo_sb = load_w(wo)

        # ================= phase 2: q proj, denom, Z, hidden =================
        for c in range(NCH):
            s0 = c * CS
            xq_c = xpool.tile([128, NKT, CS], F32R, tag="x")
            nc.sync.dma_start(out=xq_c, in_=xq[:, s0:s0 + CS].rearrange("(kt p) s -> p kt s", p=128))

            d_ps = pp_s16.tile([16, CS], F32, tag="s16")
            gT = xpool.tile([128, NMT, CS], F32R, tag="x")

            for mt in range(NMT):
                msl = bass.ts(mt, 128)
                pq = pp_proj.tile([128, CS], F32, tag="proj")
                for kt in range(NKT):
                    nc.tensor.matmul(pq, wq_sb[:, kt, msl], xq_c[:, kt, :],
                                     start=(kt == 0), stop=(kt == NKT - 1))
                # qf = elu(q)+1 (no mask on q), written into gT[:, mt, :]
                tmp = mpool.tile([128, CS], F32, tag="tmp")
                nc.vector.tensor_scalar_min(tmp, pq, 0.0)
                ex = mpool.tile([128, CS], F32, tag="ex")
                nc.scalar.activation(ex, tmp, mybir.ActivationFunctionType.Exp)
                nc.vector.scalar_tensor_tensor(gT[:, mt, :], pq, 0.0, ex,
                                               op0=AluOpType.max, op1=AluOpType.add)
                # denom head-sum of qf * Kc
                kc_sb = mpool.tile([128, CS], F32, tag="kc")
                nc.sync.dma_start(out=kc_sb, in_=kc_s[mt * 128:(mt + 1) * 128, s0:s0 + CS])
                P2 = mpool.tile([128, CS], F32R, tag="P")
                nc.vector.tensor_tensor(out=P2, in0=gT[:, mt, :], in1=kc_sb,
                                        op=AluOpType.mult)
                nc.tensor.matmul(d_ps, hs_sb[:, mt * 16:(mt + 1) * 16], P2,
                                 start=(mt == 0), stop=(mt == NMT - 1))

            # Z*C
            zt = spool.tile([16, CS], F32, tag="zt")
            nc.vector.tensor_scalar_add(zt, d_ps, EPS)
            zr = spool.tile([16, CS], F32, tag="zr")
            nc.vector.reciprocal(zr, zt)
            zc = spool.tile([16, CS], F32R, tag="zc")
            nc.vector.tensor_scalar_mul(zc, zr, c_sb[:, 0:1])

            # gT := qf * (Z*C) broadcast per head, in place
            for mt in range(NMT):
                pzc = pp_aux.tile([128, CS], F32, tag="aux")
                nc.tensor.matmul(pzc, ea_sb[:, bass.ts(mt, 128)], zc, start=True, stop=True)
                nc.vector.tensor_tensor(out=gT[:, mt, :], in0=gT[:, mt, :], in1=pzc,
                                        op=AluOpType.mult)

            # hidden = gT.T @ WoT  (s-major out)
            for sti in range(NST):
                ssl = bass.ts(sti, 128)
                for j in range(2):
                    ph = pp_proj.tile([128, 512], F32, tag="proj")
                    for mt in range(NMT):
                        nc.tensor.matmul(ph, gT[:, mt, ssl], wo_sb[:, mt, bass.ts(j, 512)],
                                         start=(mt == 0), stop=(mt == NMT - 1))
                    hs_t = mpool.tile([128, 512], F32, tag="kc")
                    nc.vector.tensor_copy(hs_t, ph)
                    nc.sync.dma_start(out=h_out[s0 + sti * 128:s0 + (sti + 1) * 128,
                                                j * 512:(j + 1) * 512], in_=hs_t)

    nc.finalize()
    return nc


def _consts():
    tri = np.triu(np.ones((64, 64), np.float32))  # [p, m] 1 where p <= m
    lblock = np.zeros((128, 128), np.float32)
    lblock[:64, :64] = tri
    lblock[64:, 64:] = tri
    hsum = np.zeros((128, 8, 16), np.float32)
    for mt in range(8):
        hsum[:64, mt, 2 * mt] = 1.0
        hsum[64:, mt, 2 * mt + 1] = 1.0
    hsum = hsum.reshape(128, 128)
    eall = np.zeros((16, D), np.float32)
    for h in range(16):
        eall[h, h * 64:(h + 1) * 64] = 1.0
    ident = np.eye(128, dtype=np.float32)
    return lblock, hsum, eall, ident


def _prep_in_maps(query, key, value, mask, Wq, Wk, Wv, Wo):
    query = np.asarray(query, np.float32)
    key = np.asarray(key, np.float32)
    value = np.asarray(value, np.float32)
    Wq = np.asarray(Wq, np.float32)
    Wk = np.asarray(Wk, np.float32)
    Wv = np.asarray(Wv, np.float32)
    Wo = np.asarray(Wo, np.float32)
    mask = np.asarray(mask)

    wqT = np.ascontiguousarray(Wq.T)
    wkT = np.ascontiguousarray(Wk.T)
    wvT = np.ascontiguousarray(Wv.T)
    woT = np.ascontiguousarray(Wo.T)
    km_full = mask[:, -1, :].astype(np.float32)          # [B, S]
    wvec_full = (S - np.arange(S)).astype(np.float32)    # weight (S - s)
    lblock, hsum, eall, ident = _consts()

    in_maps = []
    for core in range(8):
        b, half = core // 2, core % 2
        sl = slice(half * S_HALF, (half + 1) * S_HALF)
        xqT = np.ascontiguousarray(query[b].T[:, sl])
        xkT = np.ascontiguousarray(key[b].T[:, sl])
        xvT = np.ascontiguousarray(value[b].T[:, sl])
        in_maps.append({
            "xq": xqT, "xk": xkT, "xv": xvT,
            "wq": wqT, "wk": wkT, "wv": wvT, "wo": woT,
            "km": km_full[b:b + 1, sl],
            "wvec": wvec_full[None, sl],
            "lblock": lblock, "hsum": hsum, "eall": eall, "ident": ident,
        })
    return in_maps


def _assemble(r):
    hidden = np.empty((B, S, D), np.float32)
    k_full = np.empty((B, H, S, DH), np.float32)
    v_full = np.empty((B, H, S, DH), np.float32)
    for core in range(8):
        b, half = core // 2, core % 2
        sl = slice(half * S_HALF, (half + 1) * S_HALF)
        hidden[b, sl, :] = r[core]["h_out"]
        k_full[b, :, sl, :] = r[core]["k_out"]
        v_full[b, :, sl, :] = r[core]["v_out"]
    return hidden, k_full, v_full


def kernel(query, key, value, mask, Wq, Wk, Wv, Wo):
    if "nc" not in _CACHE:
        _CACHE["nc"] = _build_nc()
    in_maps = _prep_in_maps(query, key, value, mask, Wq, Wk, Wv, Wo)
    res = run_bass_kernel_spmd(_CACHE["nc"], in_maps, core_ids=list(range(8)))
    return _assemble(res.results)


def run_traced(inputs):
    """test.py helper: same run but with NTFF tracing; returns BassKernelResults."""
    if "nc" not in _CACHE:
        _CACHE["nc"] = _build_nc()
    in_maps = _prep_in_maps(**inputs)
    return run_bass_kernel_spmd(_CACHE["nc"], in_maps, core_ids=list(range(8)),
                                trace=True)


# revision 11
# speedup vs baseline: 1.1728x; 1.1728x over previous
"""Trainium2 Bass kernel for nn_MultiHeadAttention_30734785970264.

Linear ("elu+1") attention, faithful to the reference:
  q/k/v projections, kf = (elu(k)+1)*km, vm = v*km,
  C[b,h]   = sum_s (S-s) * sum_d(kf*vm)          (global over s)
  Kc       = per-head cumsum of kf over feature dim
  Z[b,h,s] = 1/(sum_d qf*Kc + eps)
  out      = qf * C * Z ;  hidden = out @ Wo.T
Returns (hidden, k, v) like the reference.

Sharding: batch x sequence-half across 8 cores (all heads local); the only
cross-core dependency is the [16]-float C partial, AllReduce'd between the
two cores of each batch. Activations are fed feature-major (host transpose)
so every GEMM contracts on the partition dim; all GEMMs run as float32r.
The binary mask enters only via host-precomputed per-position vectors
(wvec*km and km); C is folded into Wo after the AllReduce so the final
hidden GEMM is dependency-free.
"""

import sys

try:
    import concourse.bass as bass  # noqa: F401
except ImportError:
    sys.path.insert(0, "/opt/trn_rl_repo")

import numpy as np
from contextlib import ExitStack

import concourse.bass as bass
import concourse.mybir as mybir
import concourse.tile as tile
from concourse import bacc
from concourse.alu_op_type import AluOpType
from concourse.bass_utils import run_bass_kernel_spmd

F32 = mybir.dt.float32
F32R = mybir.dt.float32r

B, S, H, DH, D = 4, 4096, 16, 64, 1024
EPS = 1e-6
S_HALF = S // 2          # 2048 rows per core
CS = 256                 # s-chunk
NCH = S_HALF // CS       # 8 chunks
NMT = D // 128           # 8 feature m-tiles
NKT = D // 128           # 8 contraction k-tiles
NST = CS // 128          # 2 s-subtiles per chunk

_CACHE = {}


def _build_nc():
    nc = bacc.Bacc()

    xq = nc.declare_dram_parameter("xq", [D, S_HALF], F32R, isOutput=False)
    xk = nc.declare_dram_parameter("xk", [D, S_HALF], F32R, isOutput=False)
    xv = nc.declare_dram_parameter("xv", [D, S_HALF], F32R, isOutput=False)
    wq = nc.declare_dram_parameter("wq", [D, D], F32R, isOutput=False)
    wk = nc.declare_dram_parameter("wk", [D, D], F32R, isOutput=False)
    wv = nc.declare_dram_parameter("wv", [D, D], F32R, isOutput=False)
    wo = nc.declare_dram_parameter("wo", [D, D], F32R, isOutput=False)
    km = nc.declare_dram_parameter("km", [1, S_HALF], F32, isOutput=False)
    wvm = nc.declare_dram_parameter("wvm", [1, S_HALF], F32, isOutput=False)
    lblock = nc.declare_dram_parameter("lblock", [128, 128], F32R, isOutput=False)
    hsum = nc.declare_dram_parameter("hsum", [128, 128], F32R, isOutput=False)
    eall = nc.declare_dram_parameter("eall", [16, D], F32R, isOutput=False)

    k_out = nc.declare_dram_parameter("k_out", [D, S_HALF], F32, isOutput=True)
    v_out = nc.declare_dram_parameter("v_out", [D, S_HALF], F32, isOutput=True)
    h_out = nc.declare_dram_parameter("h_out", [S_HALF, D], F32, isOutput=True)

    gz_s = nc.dram_tensor("gz_s", [D, S_HALF], F32R)
    cc_in = nc.dram_tensor("cc_in", [16, 1], F32)
    cc_out = nc.dram_tensor("cc_out", [16, 1], F32)

    def bcast16(param, s0, n):
        t = param.tensor if hasattr(param, "tensor") else param
        return bass.AP(tensor=t, offset=s0, ap=[[0, 16], [1, n]])

    with tile.TileContext(nc) as tc, ExitStack() as ctx:
        ctx.enter_context(nc.allow_low_precision("fp32r rounding of intermediates is intentional"))
        wpool = ctx.enter_context(tc.tile_pool(name="wpool", bufs=3))
        xpool = ctx.enter_context(tc.tile_pool(name="xpool", bufs=6))
        mpool = ctx.enter_context(tc.tile_pool(name="mpool", bufs=2))
        spool = ctx.enter_context(tc.tile_pool(name="spool", bufs=2))
        cpool = ctx.enter_context(tc.tile_pool(name="cpool", bufs=1))
        pp_proj = ctx.enter_context(tc.tile_pool(name="pp_proj", bufs=4, space="PSUM"))
        pp_aux = ctx.enter_context(tc.tile_pool(name="pp_aux", bufs=2, space="PSUM"))
        pp_s16 = ctx.enter_context(tc.tile_pool(name="pp_s16", bufs=2, space="PSUM"))

        # ---- constants ----
        lb_sb = cpool.tile([128, 128], F32R, tag="lb")
        nc.sync.dma_start(out=lb_sb, in_=lblock[:, :])
        hs_sb = cpool.tile([128, 128], F32R, tag="hs")
        nc.sync.dma_start(out=hs_sb, in_=hsum[:, :])
        ea_sb = cpool.tile([16, D], F32R, tag="ea")
        nc.sync.dma_start(out=ea_sb, in_=eall[:, :])
        cacc = cpool.tile([16, 1], F32, tag="cacc")
        nc.vector.memset(cacc, 0.0)

        def load_w(param):
            t = wpool.tile([128, NKT, D], F32R, tag="w")
            v = param.rearrange("(kt p) m -> p kt m", p=128)
            for kt in range(NKT):
                nc.sync.dma_start(out=t[:, kt, :], in_=v[:, kt, :])
            return t

        wq_sb = load_w(wq)
        wk_sb = load_w(wk)
        wv_sb = load_w(wv)

        Exp = mybir.ActivationFunctionType.Exp

        # ============ phase A: projections, kf/qf, t, C, Kc, denom, gZ ============
        for c in range(NCH):
            s0 = c * CS

            def load_x(param):
                t = xpool.tile([128, NKT, CS], F32R, tag="x")
                v = param[:, s0:s0 + CS].rearrange("(kt p) s -> p kt s", p=128)
                for kt in range(NKT):
                    nc.sync.dma_start(out=t[:, kt, :], in_=v[:, kt, :])
                return t

            xq_c = load_x(xq)
            xk_c = load_x(xk)
            xv_c = load_x(xv)

            t_ps = pp_s16.tile([16, CS], F32, tag="s16")
            d_ps = pp_s16.tile([16, CS], F32, tag="s16")
            qf_big = xpool.tile([128, NMT, CS], F32R, tag="x")

            for mt in range(NMT):
                msl = bass.ts(mt, 128)
                pk = pp_proj.tile([128, CS], F32, tag="proj")
                for kt in range(NKT):
                    nc.tensor.matmul(pk, wk_sb[:, kt, msl], xk_c[:, kt, :],
                                     start=(kt == 0), stop=(kt == NKT - 1))
                kT = mpool.tile([128, CS], F32R, tag="kT")
                nc.scalar.copy(kT, pk)
                nc.sync.dma_start(out=k_out[mt * 128:(mt + 1) * 128, s0:s0 + CS],
                                  in_=kT.bitcast(F32))

                pv = pp_proj.tile([128, CS], F32, tag="proj")
                for kt in range(NKT):
                    nc.tensor.matmul(pv, wv_sb[:, kt, msl], xv_c[:, kt, :],
                                     start=(kt == 0), stop=(kt == NKT - 1))
                vT = mpool.tile([128, CS], F32R, tag="vT")
                nc.scalar.copy(vT, pv)
                nc.sync.dma_start(out=v_out[mt * 128:(mt + 1) * 128, s0:s0 + CS],
                                  in_=vT.bitcast(F32))

                pq = pp_proj.tile([128, CS], F32, tag="proj")
                for kt in range(NKT):
                    nc.tensor.matmul(pq, wq_sb[:, kt, msl], xq_c[:, kt, :],
                                     start=(kt == 0), stop=(kt == NKT - 1))

                # kf = elu(k)+1 (mask folded into wvm / Z), qf = elu(q)+1
                tmpk = mpool.tile([128, CS], F32, tag="tmpk")
                nc.vector.tensor_scalar_min(tmpk, kT, 0.0)
                exk = mpool.tile([128, CS], F32, tag="exk")
                nc.scalar.activation(exk, tmpk, Exp)
                kf = mpool.tile([128, CS], F32R, tag="kf")
                nc.vector.scalar_tensor_tensor(kf, kT, 0.0, exk,
                                               op0=AluOpType.max, op1=AluOpType.add)

                tmpq = mpool.tile([128, CS], F32, tag="tmpq")
                nc.vector.tensor_scalar_min(tmpq, pq, 0.0)
                exq = mpool.tile([128, CS], F32, tag="exq")
                nc.scalar.activation(exq, tmpq, Exp)
                nc.vector.scalar_tensor_tensor(qf_big[:, mt, :], pq, 0.0, exq,
                                               op0=AluOpType.max, op1=AluOpType.add)

                # t[h,s] += headsum(kf * v)
                P0 = mpool.tile([128, CS], F32R, tag="P0")
                nc.vector.tensor_tensor(out=P0, in0=kf, in1=vT, op=AluOpType.mult)
                nc.tensor.matmul(t_ps, hs_sb[:, mt * 16:(mt + 1) * 16], P0,
                                 start=(mt == 0), stop=(mt == NMT - 1))

                # denom[h,s] += headsum(qf * Kc)
                pkc = pp_aux.tile([128, CS], F32, tag="aux")
                nc.tensor.matmul(pkc, lb_sb, kf, start=True, stop=True)
                P2 = mpool.tile([128, CS], F32R, tag="P2")
                nc.vector.tensor_tensor(out=P2, in0=qf_big[:, mt, :], in1=pkc,
                                        op=AluOpType.mult)
                nc.tensor.matmul(d_ps, hs_sb[:, mt * 16:(mt + 1) * 16], P2,
                                 start=(mt == 0), stop=(mt == NMT - 1))

            # C partial: sum_s t[h,s] * (wvec*km)[s]
            wv16 = spool.tile([16, CS], F32, tag="wv16")
            nc.sync.dma_start(out=wv16, in_=bcast16(wvm, s0, CS))
            tmul = spool.tile([16, CS], F32, tag="tmul")
            nc.vector.tensor_tensor(out=tmul, in0=t_ps, in1=wv16, op=AluOpType.mult)
            cch = spool.tile([16, 1], F32, tag="cch")
            nc.vector.tensor_reduce(cch, tmul, axis=mybir.AxisListType.X, op=AluOpType.add)
            nc.vector.tensor_add(cacc, cacc, cch)

            # Z = 1/(denom*km + eps)  (C-free), gZ = qf * Z  (in place)
            km16 = spool.tile([16, CS], F32, tag="km16")
            nc.sync.dma_start(out=km16, in_=bcast16(km, s0, CS))
            d1 = spool.tile([16, CS], F32, tag="d1")
            nc.vector.tensor_tensor(out=d1, in0=d_ps, in1=km16, op=AluOpType.mult)
            zt = spool.tile([16, CS], F32, tag="zt")
            nc.vector.tensor_scalar_add(zt, d1, EPS)
            zr = spool.tile([16, CS], F32R, tag="zr")
            nc.vector.reciprocal(zr, zt)
            for mt in range(NMT):
                pzr = pp_aux.tile([128, CS], F32, tag="aux")
                nc.tensor.matmul(pzr, ea_sb[:, bass.ts(mt, 128)], zr, start=True, stop=True)
                nc.vector.tensor_tensor(out=qf_big[:, mt, :], in0=qf_big[:, mt, :],
                                        in1=pzr, op=AluOpType.mult)
            nc.sync.dma_start(
                out=gz_s[:, s0:s0 + CS].rearrange("(kt p) s -> p kt s", p=128),
                in_=qf_big)

        # ---- AllReduce C across the two cores of this batch ----
        nc.sync.dma_start(out=cc_in[:, :], in_=cacc)
        nc.gpsimd.collective_compute(
            "AllReduce", AluOpType.add,
            replica_groups=[[0, 1], [2, 3], [4, 5], [6, 7]],
            ins=[cc_in.ap().opt()], outs=[cc_out.ap().opt()])
        c_sb = cpool.tile([16, 1], F32, tag="c_sb")
        nc.sync.dma_start(out=c_sb, in_=cc_out[:, :])
        ones16 = cpool.tile([16, 8], F32, tag="ones16")
        nc.vector.memset(ones16, 1.0)
        c_sbr = cpool.tile([16, 8], F32R, tag="c_sbr")
        nc.vector.tensor_scalar_mul(c_sbr, ones16, c_sb[:, 0:1])

        # ============ phase B: wo' = wo * C, hidden = gZ.T @ wo' ============
        wo_sb = load_w(wo)
        wo2 = wpool.tile([128, NKT, D], F32R, tag="w")
        for kt in range(NKT):
            pce = pp_aux.tile([128, 8], F32, tag="aux")
            nc.tensor.matmul(pce, ea_sb[:, bass.ts(kt, 128)], c_sbr, start=True, stop=True)
            nc.vector.tensor_scalar_mul(wo2[:, kt, :], wo_sb[:, kt, :], pce[:, 0:1])

        for c in range(NCH):
            s0 = c * CS
            gz_c = xpool.tile([128, NMT, CS], F32R, tag="x")
            v = gz_s[:, s0:s0 + CS].rearrange("(kt p) s -> p kt s", p=128)
            for kt in range(NKT):
                nc.sync.dma_start(out=gz_c[:, kt, :], in_=v[:, kt, :])
            for sti in range(NST):
                ssl = bass.ts(sti, 128)
                for j in range(2):
                    ph = pp_proj.tile([128, 512], F32, tag="proj")
                    for mt in range(NMT):
                        nc.tensor.matmul(ph, gz_c[:, mt, ssl], wo2[:, mt, bass.ts(j, 512)],
                                         start=(mt == 0), stop=(mt == NMT - 1))
                    hs_t = mpool.tile([128, 512], F32, tag="hs_t")
                    nc.vector.tensor_copy(hs_t, ph)
                    nc.sync.dma_start(out=h_out[s0 + sti * 128:s0 + (sti + 1) * 128,
                                                j * 512:(j + 1) * 512], in_=hs_t)

    nc.finalize()
    return nc


def _consts():
    tri = np.triu(np.ones((64, 64), np.float32))  # [p, m] 1 where p <= m
    lblock = np.zeros((128, 128), np.float32)
    lblock[:64, :64] = tri
    lblock[64:, 64:] = tri
    hsum = np.zeros((128, 8, 16), np.float32)
    for mt in range(8):
        hsum[:64, mt, 2 * mt] = 1.0
        hsum[64:, mt, 2 * mt + 1] = 1.0
    hsum = hsum.reshape(128, 128)
    eall = np.zeros((16, D), np.float32)
    for h in range(16):
        eall[h, h * 64:(h + 1) * 64] = 1.0
    return lblock, hsum, eall


def _prep_in_maps(query, key, value, mask, Wq, Wk, Wv, Wo):
    query = np.asarray(query, np.float32)
    key = np.asarray(key, np.float32)
    value = np.asarray(value, np.float32)
    Wq = np.asarray(Wq, np.float32)
    Wk = np.asarray(Wk, np.float32)
    Wv = np.asarray(Wv, np.float32)
    Wo = np.asarray(Wo, np.float32)
    mask = np.asarray(mask)

    wqT = np.ascontiguousarray(Wq.T)
    wkT = np.ascontiguousarray(Wk.T)
    wvT = np.ascontiguousarray(Wv.T)
    woT = np.ascontiguousarray(Wo.T)
    km_full = mask[:, -1, :].astype(np.float32)          # [B, S]
    wvec_full = (S - np.arange(S)).astype(np.float32)    # weight (S - s)
    lblock, hsum, eall = _consts()

    in_maps = []
    for core in range(8):
        b, half = core // 2, core % 2
        sl = slice(half * S_HALF, (half + 1) * S_HALF)
        xqT = np.ascontiguousarray(query[b].T[:, sl])
        xkT = np.ascontiguousarray(key[b].T[:, sl])
        xvT = np.ascontiguousarray(value[b].T[:, sl])
        km_c = km_full[b:b + 1, sl]
        in_maps.append({
            "xq": xqT, "xk": xkT, "xv": xvT,
            "wq": wqT, "wk": wkT, "wv": wvT, "wo": woT,
            "km": np.ascontiguousarray(km_c),
            "wvm": np.ascontiguousarray(wvec_full[None, sl] * km_c),
            "lblock": lblock, "hsum": hsum, "eall": eall,
        })
    return in_maps


def _assemble(r):
    hidden = np.empty((B, S, D), np.float32)
    k_full = np.empty((B, H, S, DH), np.float32)
    v_full = np.empty((B, H, S, DH), np.float32)
    for core in range(8):
        b, half = core // 2, core % 2
        sl = slice(half * S_HALF, (half + 1) * S_HALF)
        hidden[b, sl, :] = r[core]["h_out"]
        k_full[b, :, sl, :] = r[core]["k_out"].reshape(H, DH, S_HALF).transpose(0, 2, 1)
        v_full[b, :, sl, :] = r[core]["v_out"].reshape(H, DH, S_HALF).transpose(0, 2, 1)
    return hidden, k_full, v_full


def kernel(query, key, value, mask, Wq, Wk, Wv, Wo):
    if "nc" not in _CACHE:
        _CACHE["nc"] = _build_nc()
    in_maps = _prep_in_maps(query, key, value, mask, Wq, Wk, Wv, Wo)
    res = run_bass_kernel_spmd(_CACHE["nc"], in_maps, core_ids=list(range(8)))
    return _assemble(res.results)


def run_traced(inputs):
    """test.py helper: same run but with NTFF tracing; returns BassKernelResults."""
    if "nc" not in _CACHE:
        _CACHE["nc"] = _build_nc()
    in_maps = _prep_in_maps(**inputs)
    return run_bass_kernel_spmd(_CACHE["nc"], in_maps, core_ids=list(range(8)),
                                trace=True)
